# revision 27
# baseline (speedup 1.0000x reference)
"""MLA (multi-head latent attention) Bass kernel for Trainium2, 8 NeuronCores.

Sharding: data-parallel over batch (cores 0-3 = batch 0, cores 4-7 = batch 1),
tensor-parallel over heads within each group (4 of 16 heads per core).

v6: seq-chunk pipelined design.
 - The sequence is processed in 4 chunks of 512. Latent projections (P) run
   first (DMA-bound on the x stream); the kv-latent AllGathers (pairwise
   merged, bf16) overlap them. Value up-projection (V) and causal attention
   (A) follow per-chunk, interleaved so the PE's static instruction order
   never waits on a collective that hasn't had time to complete.
 - The output projection runs BEFORE the second collective: each core
   computes the full-7168-row partial product with its own 4 heads' attention
   outputs (WOp), then a bf16 ReduceScatter sums the 4 partials and scatters
   the 1792-row output slice to each core. This keeps the collective payload
   small (RS output is 1/4 of an AllGather's) and leaves only a DRAM->DRAM
   copy after the last collective instead of a full matmul stage.
 - Collective-consumer DMAs (kv gather loads, RS output copy-out) issue from
   the gpsimd queue, which blocks on in-flight collectives anyway; putting
   them on a compute engine's HWDGE ring lets the scheduler hoist them ahead
   of compute, stalling that engine on the collective semaphore.
 - bf16 everywhere except attention scores/softmax (fp32) and the psum
   accumulators (always fp32).
 - Softmax denominators accumulate on the vector engine (one fold matmul per
   head/chunk instead of one per key block); the reciprocal row is broadcast
   across partitions with a PE outer product.
"""

import numpy as np
import ml_dtypes

import concourse.bacc as bacc
import concourse.bass as bass
import concourse.mybir as mybir
import concourse.tile as tile
from concourse.bass_utils import run_bass_kernel_spmd

# Problem constants (nn_MLA_50379966382638)
B, S, D = 2, 2048, 7168
R, H, VD = 1024, 16, 128
QK_HD = R // H            # 64
SCALE = float(np.sqrt(D // H))

N_CORES = 8
TP = 4                    # tensor-parallel ranks per batch group
HPC = H // TP             # 4 heads per core
RS = R // TP              # 256 latent dims per core
VS = HPC * VD             # 512 value dims per core
DS = D // TP              # 1792 output dims per core
GROUPS = [[0, 1, 2, 3], [4, 5, 6, 7]]

DM = D // 128             # 56 d_model chunks
SQ = S // 512             # 4 seq chunks of 512
XG = 8                    # x subgroups per seq chunk (7 dm chunks each)
XGD = DM // XG            # 7
DO = DS // 128            # 14 output-dim chunks per core
HV = (H * VD) // 128      # 16 hv chunks

F32 = mybir.dt.float32
F32R = mybir.dt.float32r
BF16 = mybir.dt.bfloat16
EXP = mybir.ActivationFunctionType.Exp
BNP = ml_dtypes.bfloat16

_CACHE = {}


def _emit(nc, tc, xT, wqkvT, wvbT, woR, maskT, ones, outT):
    ts = bass.ts
    SC = 1.0 / SCALE

    with (
        tc.tile_pool(name="const", bufs=1) as cpool,
        tc.tile_pool(name="lat", bufs=1) as lat_pool,
        tc.tile_pool(name="kvbp", bufs=2) as kvb_pool,
        tc.tile_pool(name="kvfp", bufs=2) as kvf_pool,
        tc.tile_pool(name="dram", bufs=1, space="DRAM") as dram_pool,
        tc.tile_pool(name="psA", bufs=1, space="PSUM") as psA,
        tc.tile_pool(name="psc", bufs=2, space="PSUM") as psc,
        tc.tile_pool(name="psv", bufs=2, space="PSUM") as psv,
    ):
        # constants: one shifted causal mask [128, 896]; ones col for denom fold
        mask_t = cpool.tile([128, 896], F32R, tag="mask", name="mask")
        ones_t = cpool.tile([128, 128], F32R, tag="ones", name="ones_t")
        ones_col = ones_t[:, 0:1]
        ones_row = ones_t[0:1, :]

        # full-seq latents, [128 latent, S] per tile
        qlat = [lat_pool.tile([128, S], F32R, tag=f"qlat{i}", name=f"qlat{i}")
                for i in range(2)]
        kvlat = [lat_pool.tile([128, S], F32R, tag=f"kvlat{i}", name=f"kvlat{i}")
                 for i in range(2)]
        # v in [seq, vd] layout, one tile per 128-seq block (alloc post-P)
        v_t = []
        # attention outputs stay in SBUF until the partial output projection
        aoc_t = {}

        # collective bounce buffers
        kvbi = [dram_pool.tile([RS, 512], BF16, tag=f"kvbi{g}", name=f"kvbi{g}")
                for g in range(SQ)]
        kvbo = [dram_pool.tile([R, 512], BF16, tag=f"kvbo{g}", name=f"kvbo{g}")
                for g in range(SQ)]
        atbi = [dram_pool.tile([VS, 512], BF16, tag=f"atbi{q}", name=f"atbi{q}")
                for q in range(SQ)]
        atbo = [dram_pool.tile([H * VD, 512], BF16, tag=f"atbo{q}", name=f"atbo{q}")
                for q in range(SQ)]
        atf_t = []

        kvf2 = []

        def emit_P(qc):
            accs = [psA.tile([128, 512], F32, tag=f"p{j}", name=f"pacc{j}_{qc}")
                    for j in range(4)]
            for g in range(XG):
                if qc == 0:
                    wg = w_pool.tile([128, XGD * 512], BF16, tag=f"wqkv{g}",
                                     name=f"wqkv{g}")
                    nc.sync.dma_start(
                        wg[:].rearrange("p (c j) -> p c j", c=XGD),
                        wqkvT[ts(g, XGD * 128), :]
                        .rearrange("(c p) j -> p c j", p=128))
                    wqkv_g.append(wg)
                xt = x_pool.tile([128, XGD * 512], BF16, tag="xt",
                                 name=f"xt{qc}_{g}")
                nc.sync.dma_start(
                    xt[:].rearrange("p (c j) -> p c j", c=XGD),
                    xT[ts(g, XGD * 128), ts(qc, 512)]
                    .rearrange("(c p) j -> p c j", p=128))
                for c in range(XGD):
                    dm = XGD * g + c
                    mv = xt[:, ts(c, 512)]
                    st, sp = dm == 0, dm == DM - 1
                    for j in range(4):
                        nc.tensor.matmul(
                            accs[j][:],
                            wqkv_g[g][:, c * 512 + j * 128: c * 512 + (j + 1) * 128],
                            mv, start=st, stop=sp)
            nc.scalar.copy(qlat[0][:, ts(qc, 512)], accs[0][:])
            nc.scalar.copy(qlat[1][:, ts(qc, 512)], accs[1][:])
            nc.vector.tensor_copy(kvlat[0][:, ts(qc, 512)], accs[2][:])
            nc.vector.tensor_copy(kvlat[1][:, ts(qc, 512)], accs[3][:])
            for i in range(2):
                kvb = kvb_pool.tile([128, 512], BF16, tag="kvb16",
                                    name=f"kvb16_{qc}_{i}")
                with nc.allow_low_precision(reason="bf16 kv-latent allgather"):
                    if i == 0:
                        nc.scalar.copy(kvb[:], accs[2 + i][:])
                    else:
                        nc.vector.tensor_copy(kvb[:], accs[2 + i][:])
                nc.sync.dma_start(kvbi[qc][ts(i, 128), :], kvb[:])

        def emit_AG1(g):
            nc.gpsimd.collective_compute(
                "AllGather", mybir.AluOpType.bypass, replica_groups=GROUPS,
                ins=[kvbi[g][:].opt()], outs=[kvbo[g][:].opt()])


        def emit_V(qc):
            kvf = kvf2[qc]
            for ss in range(4):
                kb = 4 * qc + ss
                acc = psA.tile([128, VS], F32, tag=f"p{ss}", name=f"vacc{kb}")
                for lc in range(8):
                    base = lc * 512 + ss * 128
                    nc.tensor.matmul(
                        acc[:], kvf[:, base: base + 128],
                        wvb_all[:, ts(lc, 512)], start=lc == 0, stop=lc == 7)
                if ss % 2 == 0:
                    nc.scalar.copy(v_t[kb][:], acc[:])
                else:
                    nc.vector.tensor_copy(v_t[kb][:], acc[:])

        def emit_A(qc, pairs, rng, st):
            nkc = 4 * qc + 4
            qs = ts(qc, 512)
            for pair in pairs:
                # heads 2*pair (latent rows 0-63) and 2*pair+1 (rows 64-127)
                # share tile `pair`; their score matmuls contract disjoint
                # row groups
                if pair not in st:
                    av_p, exsum_p = [], []
                    for m in range(2):
                        h = 2 * pair + m
                        av = psv.tile([128, 512], F32, tag="av",
                                      name=f"av{h}_{qc}")
                        exsum = att2.tile([128, 512], F32R, tag="exsum",
                                          name=f"exsum{h}_{qc}")
                        av_p.append(av)
                        exsum_p.append(exsum)
                    st[pair] = (av_p, exsum_p)
                av_p, exsum_p = st[pair]
                for kc in rng:
                    exs = []
                    for m in range(2):
                        r0 = m * 64
                        scp = psc.tile([128, 512], F32, tag="sc",
                                       name=f"sc{pair}_{m}_{qc}_{kc}")
                        nc.tensor.matmul(
                            scp[:],
                            kvlat[pair][r0:r0 + 64, ts(kc, 128)],
                            qlat[pair][r0:r0 + 64, qs],
                            start=True, stop=True)
                        ex = ex_pool.tile([128, 512], F32R, tag="ex",
                                          name=f"ex{pair}_{m}_{qc}_{kc}")
                        nc.scalar.activation(ex[:], scp[:], EXP, scale=SC)
                        exs.append(ex)
                    j = kc - 4 * qc
                    for m in range(2):
                        ex = exs[m]
                        if j >= 0:
                            nc.vector.tensor_mul(
                                ex[:], ex[:],
                                mask_t[:, 384 - 128 * j: 896 - 128 * j])
                        if kc == 0:
                            nc.vector.tensor_copy(exsum_p[m][:], ex[:])
                        else:
                            nc.vector.tensor_add(exsum_p[m][:], exsum_p[m][:], ex[:])
                        h = 2 * pair + m
                        nc.tensor.matmul(av_p[m][:], v_t[kc][:, ts(h, 128)],
                                         ex[:], start=kc == 0, stop=kc == nkc - 1)

        def emit_A_fin(qc, pairs, st):
            for pair in pairs:
                av_p, exsum_p = st[pair]
                for m in range(2):
                    h = 2 * pair + m
                    smp = psc.tile([1, 512], F32, tag="sc", name=f"sm{h}_{qc}")
                    nc.tensor.matmul(smp[:], ones_col, exsum_p[m][:],
                                     start=True, stop=True)
                    rc = att2.tile([1, 512], F32R, tag="rc", name=f"rc{h}_{qc}")
                    with nc.allow_low_precision(reason="f32r is bit-identical to f32"):
                        nc.vector.reciprocal(rc[:], smp[:])
                    bcs = att2.tile([128, 512], F32R, tag="bcs",
                                    name=f"bcs{h}_{qc}")
                    nc.gpsimd.partition_broadcast(bcs[:], rc[:])
                    aoc = aoc_pool.tile([128, 512], BF16, tag="aoc",
                                        name=f"aoc{h}_{qc}")
                    with nc.allow_low_precision(reason="bf16 attn output"):
                        nc.vector.tensor_mul(aoc[:], av_p[m][:], bcs[:])
                    nc.sync.dma_start(atbi[qc][ts(h, 128), :], aoc[:])

        def emit_AG2(qc):
            nc.gpsimd.collective_compute(
                "AllGather", mybir.AluOpType.bypass, replica_groups=GROUPS,
                ins=[atbi[qc][:].opt()], outs=[atbo[qc][:].opt()])


        # ---- emission ----
        with (
            tc.tile_pool(name="wproj", bufs=1) as w_pool,
            tc.tile_pool(name="xs", bufs=2) as x_pool,
        ):
            wqkv_g = []
            emit_P(0)
            nc.sync.dma_start(mask_t[:], maskT[:])
            nc.sync.dma_start(ones_t[:], ones[:])
            emit_AG1(0)
            emit_P(1)
            emit_AG1(1)
            emit_P(2)
            emit_AG1(2)
            emit_P(3)
            emit_AG1(3)

        with (
            tc.tile_pool(name="vsb", bufs=1) as v_pool,
            tc.tile_pool(name="ex", bufs=3) as ex_pool,
            tc.tile_pool(name="att2", bufs=2) as att2,
            tc.tile_pool(name="aocp", bufs=4) as aoc_pool,
            tc.tile_pool(name="wvbp", bufs=1) as wvb_pool,
            tc.tile_pool(name="wop", bufs=1) as wod_pool,
            tc.tile_pool(name="atfp", bufs=1) as atf_pool,
            tc.tile_pool(name="rsp", bufs=2) as rs_pool,
        ):
            v_t.extend(v_pool.tile([128, VS], F32R, tag=f"v{k}", name=f"v{k}")
                       for k in range(S // 128))
            for g in range(SQ):
                kvf = kvf_pool.tile([128, 8 * 512], BF16, tag="kvf",
                                    name=f"kvf{g}")
                nc.sync.dma_start(
                    kvf[:].rearrange("p (c j) -> p c j", c=8),
                    kvbo[g][:].rearrange("(c p) j -> p c j", p=128))
                kvf2.append(kvf)
            wvb_all = wvb_pool.tile([128, 8 * 512], BF16, tag="wvb", name="wvb")
            nc.sync.dma_start(
                wvb_all[:].rearrange("p (c j) -> p c j", c=8),
                wvbT[:].rearrange("(c p) j -> p c j", p=128))
            wods = [wod_pool.tile([128, HV * 128], BF16, tag=f"wod{d}",
                                  name=f"wod{d}") for d in range(DO)]
            for d in range(DO):
                nc.sync.dma_start(wods[d][:], woR[:, ts(d, HV * 128)])

            emit_V(0)
            st = {}
            emit_A(0, [0], range(0, 4), st)
            emit_A_fin(0, [0], st)
            emit_A(0, [1], range(0, 4), st)
            emit_A_fin(0, [1], st)
            emit_AG2(0)
            for qc in range(1, SQ):
                # pair0's prior-chunk blocks don't need v(qc): run them while
                # the kv AllGather may still be in flight, then V(qc), the
                # rest
                nkc = 4 * qc + 4
                st = {}
                emit_A(qc, [0], range(0, 4 * qc), st)
                emit_V(qc)
                emit_A(qc, [0], range(4 * qc, nkc), st)
                emit_A_fin(qc, [0], st)
                emit_A(qc, [1], range(0, nkc), st)
                emit_A_fin(qc, [1], st)
                emit_AG2(qc)
            for qc in range(SQ):
                atf = atf_pool.tile([128, HV * 512], BF16, tag="atf",
                                    name=f"atf{qc}")
                nc.sync.dma_start(
                    atf[:].rearrange("p (c j) -> p c j", c=HV),
                    atbo[qc][:].rearrange("(c p) j -> p c j", p=128))
                atf_t.append(atf)
                for d in range(DO):
                    acc = psA.tile([128, 512], F32, tag=f"p{d % 4}",
                                   name=f"oacc{d}_{qc}")
                    for c in range(HV):
                        nc.tensor.matmul(acc[:], wods[d][:, ts(c, 128)],
                                         atf_t[qc][:, ts(c, 512)],
                                         start=c == 0, stop=c == HV - 1)
                    ot = rs_pool.tile([128, 512], F32, tag="ot",
                                      name=f"ot{d}_{qc}")
                    if d % 2 == 0:
                        nc.scalar.copy(ot[:], acc[:])
                    else:
                        nc.vector.tensor_copy(ot[:], acc[:])
                    nc.sync.dma_start(outT[ts(d, 128), ts(qc, 512)], ot[:])


def _build():
    if "nc" in _CACHE:
        return _CACHE["nc"]
    nc = bacc.Bacc("TRN2", target_bir_lowering=False, debug=False,
                   num_devices=N_CORES)
    xT = nc.dram_tensor("xT", [D, S], BF16, kind="ExternalInput").ap()
    wqkvT = nc.dram_tensor("wqkvT", [D, 2 * RS], BF16, kind="ExternalInput").ap()
    wvbT = nc.dram_tensor("wvbT", [R, VS], BF16, kind="ExternalInput").ap()
    woR = nc.dram_tensor("woR", [128, DO * HV * 128], BF16,
                         kind="ExternalInput").ap()
    maskT = nc.dram_tensor("maskT", [128, 896], F32R, kind="ExternalInput").ap()
    ones = nc.dram_tensor("ones", [128, 128], F32R, kind="ExternalInput").ap()
    outT = nc.dram_tensor("outT", [DS, S], F32, kind="ExternalOutput").ap()
    with tile.TileContext(nc) as tc:
        _emit(nc, tc, xT, wqkvT, wvbT, woR, maskT, ones, outT)
    nc.compile()
    _CACHE["nc"] = nc
    return nc


def _host_mask():
    p = np.arange(128, dtype=np.float32)[:, None]
    f = np.arange(896, dtype=np.float32)[None, :]
    return (p <= f - 384).astype(np.float32)


def _in_maps(inputs):
    x = np.asarray(inputs["x"], dtype=np.float32)
    Wq = np.asarray(inputs["Wq"], np.float32)
    Wkv = np.asarray(inputs["Wkv"], np.float32)
    Wvb = np.asarray(inputs["Wvb"], np.float32)
    Wo = np.asarray(inputs["Wo"], np.float32)
    maskT = _host_mask()
    onesm = np.ones((128, 128), np.float32)
    xTs = [np.ascontiguousarray(x[g].astype(BNP).T) for g in range(B)]
    in_maps = []
    for c in range(N_CORES):
        g, t = c // TP, c % TP
        wqkv = np.concatenate(
            [Wq[t * RS:(t + 1) * RS, :], Wkv[t * RS:(t + 1) * RS, :]], axis=0)
        # Wo row shard [DS, H*VD] -> woR[p, (d, c, i)] = Wo[t*DS + d*128+i,
        # c*128 + p]
        wo = Wo[t * DS:(t + 1) * DS, :].astype(BNP)
        woR = np.ascontiguousarray(
            wo.reshape(DO, 128, HV, 128).transpose(3, 0, 2, 1).reshape(
                128, DO * HV * 128))
        in_maps.append({
            "xT": xTs[g],
            "wqkvT": np.ascontiguousarray(wqkv.astype(BNP).T),
            "wvbT": np.ascontiguousarray(Wvb[t * VS:(t + 1) * VS, :].astype(BNP).T),
            "woR": woR,
            "maskT": maskT,
            "ones": onesm,
        })
    return in_maps


def _assemble(results, bo):
    bo = np.asarray(bo, np.float32)
    out = np.empty((B, S, D), dtype=np.float32)
    for c in range(N_CORES):
        g, t = c // TP, c % TP
        out[g, :, t * DS:(t + 1) * DS] = results[c]["outT"].T
    if bo.any():
        out += bo
    return out


def kernel(x, Wq, Wkv, Wvb, Wo, bo):
    nc = _build()
    in_maps = _in_maps(dict(x=x, Wq=Wq, Wkv=Wkv, Wvb=Wvb, Wo=Wo))
    res = run_bass_kernel_spmd(nc, in_maps, core_ids=list(range(N_CORES)))
    return _assemble(res.results, bo)
